# revision 25
# baseline (speedup 1.0000x reference)
"""Trainium2 Bass kernel for nn_AGCRN_Model (gnn_message_passing).

Self-contained: call kernel(**inputs) with the full reference inputs.

Algorithm (reference simplifies because H==0 throughout):
  per layer L: pre = A_norm @ x @ Wcat_L + A_norm-agg(eattr @ Wecat_L) + b_L
               h   = (1-sigmoid(pre_R)) * tanh(pre_U)   [relu after layer 0]
Sharding: by destination node. Core c owns 1280 node slots (10 blocks x 128),
all 12 timesteps. Host relabels nodes (degree-balanced blocks) and bakes the
GCN norm rs[dst]*rs[src] into one-hot scatter matrices sc.

v2 design (vs baseline):
- Layer 0 does NO device gather: the host pre-stages x rows in edge order
  (xe table, [128, NBLK*C, 192] bf16) so the device streams it contiguously.
- The edge-feature term (sum_e norm*ea @ Wecat + bias) is precomputed on the
  host into per-block prebase tables (T-independent static graph prep).
- h exchange is a pipelined sequence of per-block AllGathers into a
  block-major h_table (row = src_blk*1024 + src_core*128 + off), each issued
  right after its L0 block finishes; a tiny warmup collective at t~0 absorbs
  the cc bootstrap barrier.
- Layer-1 gathers are split into 3 chunk-aligned sub-gathers per block,
  edges sorted by source block so each sub-gather only needs the first few
  AllGather chunks (gates precomputed on host, maxed across cores so one
  NEFF serves all 8 cores).
"""
import sys

sys.path.insert(0, '/opt/trn_rl_repo')

import numpy as np
import ml_dtypes

import concourse.bass as bass
import concourse.mybir as mybir
from concourse import bacc, tile
from concourse.bass_utils import run_bass_kernel_spmd

N = 10000
E = 100000
T = 12
CIN = 16
CE = 8
HID = 32
NCORES = 8
NBLK = 10
NODES_PER_CORE = NBLK * 128          # 1280
N_PAD = NCORES * NODES_PER_CORE      # 10240
NSUB = 2                             # L1 sub-gathers per block
EXCHANGE = "ag2"                     # "ag1" | "ag2" (two grouped AllGathers)
GB = 5                               # L0 blocks per AllGather group (ag2)

bf16 = ml_dtypes.bfloat16
DT = mybir.dt


def _to_bf16(a):
    return np.asarray(a).astype(bf16)


# ---------------------------------------------------------------- host prep
def _host_prep(x, edge_index, edge_attr,
               Wg0, Weg0, bg0, Wu0, Weu0, bu0,
               Wg1, Weg1, bg1, Wu1, Weu1, bu1):
    X = np.asarray(x, np.float32)[0]                  # [T, N, CIN]
    src = np.asarray(edge_index[0]).astype(np.int64)
    dst = np.asarray(edge_index[1]).astype(np.int64)
    ea = np.asarray(edge_attr, np.float32)            # [E, CE]

    deg = np.maximum(np.bincount(dst, minlength=N).astype(np.float32), 1.0)
    rs = 1.0 / np.sqrt(deg)

    # --- node -> (core, block, offset): snake-deal by degree, then bin-pack
    order = np.argsort(-deg, kind='stable')
    core_of = np.empty(N, np.int64)
    for i, n in enumerate(order):
        k = i % (2 * NCORES)
        core_of[n] = k if k < NCORES else 2 * NCORES - 1 - k
    perm_slot = np.empty(N, np.int64)
    for c in range(NCORES):
        nodes_c = np.where(core_of == c)[0]
        nodes_c = nodes_c[np.argsort(-deg[nodes_c], kind='stable')]
        blk_load = np.zeros(NBLK)
        blk_fill = np.zeros(NBLK, np.int64)
        for n in nodes_c:
            cand = np.where(blk_fill < 128)[0]
            b = cand[np.argmin(blk_load[cand])]
            perm_slot[n] = c * NODES_PER_CORE + b * 128 + blk_fill[b]
            blk_fill[b] += 1
            blk_load[b] += deg[n]

    # x rows keyed by node, t-major: [N, T*CIN]
    xrow = np.ascontiguousarray(X.transpose(1, 0, 2).reshape(N, T * CIN))

    # --- edge -> dst slot / src h_table row (block-major table layout)
    dslot = perm_slot[dst]
    dcore = dslot // NODES_PER_CORE
    dblk = (dslot % NODES_PER_CORE) // 128
    doff = dslot % 128
    sslot = perm_slot[src]
    s_core = sslot // NODES_PER_CORE
    s_blk = (sslot % NODES_PER_CORE) // 128
    s_off = sslot % 128
    # group-rank-major: AllGather of blocks [g*GB,(g+1)*GB) lands rows
    # [g*GB*1024, ...) in (core, local block, off) order (GB=NBLK -> one AG,
    # plain rank-major)
    s_gate = s_blk // GB
    s_row = (s_gate * (NCORES * GB * 128) + s_core * (GB * 128)
             + (s_blk % GB) * 128 + s_off)
    enorm = rs[dst] * rs[src]

    maxblk = 0
    core_edges = []
    for c in range(NCORES):
        per_blk = []
        for b in range(NBLK):
            es = np.where((dcore == c) & (dblk == b))[0]
            # sort by (AG group, src row) so early sub-gathers only need the
            # first AllGather chunks, and reads are ~sequential in h_table
            es = es[np.lexsort((s_row[es], s_gate[es]))]
            per_blk.append(es)
            maxblk = max(maxblk, len(es))
        core_edges.append(per_blk)
    C = int(np.ceil(maxblk / 128))
    E_blk = C * 128
    E_pad = NBLK * E_blk

    # chunk-range boundaries for the NSUB sub-gathers (shared across cores);
    # first boundary below half so wave 1 usually needs only AG group 0
    if NSUB == 2:
        bnd = [0, max(1, int(np.floor(C * 0.4))), C]
    else:
        bnd = [0] + [int(np.ceil(C * (k + 1) / NSUB)) for k in range(NSUB)]
    bnd = sorted(set(bnd))  # strictly increasing, ends at C
    gates = np.zeros((NBLK, len(bnd) - 1), np.int64)  # min AG chunk needed

    per_core = []
    for c in range(NCORES):
        idx = np.zeros(E_pad, np.int16)
        grp = np.zeros(E_pad, np.int64)
        sc = np.zeros((NBLK * C, 128, 128), np.float32)   # (chunk, e_row, doff)
        xe = np.zeros((128, NBLK * C, T * CIN), np.float32)  # (e_row, chunk, :)
        for b in range(NBLK):
            es = core_edges[c][b]
            k = len(es)
            idx[b * E_blk:b * E_blk + k] = s_row[es].astype(np.int16)
            grp[b * E_blk:b * E_blk + k] = s_gate[es]
            rows = np.arange(k)
            sc[b * C + rows // 128, rows % 128, doff[es]] = enorm[es]
            xe[rows % 128, b * C + rows // 128, :] = xrow[src[es]]
            for s in range(len(bnd) - 1):
                lo, hi = b * E_blk + bnd[s] * 128, b * E_blk + bnd[s + 1] * 128
                # pad entries have grp 0 (they gather h_table row 0)
                gates[b, s] = max(gates[b, s], int(grp[lo:hi].max()))
        idx_w = np.tile(idx.reshape(-1, 16).T, (8, 1)).copy()
        sc_dev = _to_bf16(np.transpose(sc, (1, 0, 2)))    # [128, NBLK*C, 128]
        per_core.append(dict(idx=idx_w, sc=sc_dev, xe=_to_bf16(xe)))

    # --- per-slot prebase (edge-feature aggregate + bias), both layers
    Wg0, Wu0 = np.asarray(Wg0, np.float32), np.asarray(Wu0, np.float32)
    Wg1, Wu1 = np.asarray(Wg1, np.float32), np.asarray(Wu1, np.float32)
    Weg0, Weu0 = np.asarray(Weg0, np.float32), np.asarray(Weu0, np.float32)
    Weg1, Weu1 = np.asarray(Weg1, np.float32), np.asarray(Weu1, np.float32)
    agg_ea = np.zeros((N_PAD, CE), np.float32)
    np.add.at(agg_ea, dslot, ea * enorm[:, None])
    Wecat0 = np.concatenate([Weg0[:, HID:], Weu0], axis=1)   # [8, 64]
    Wecat1 = np.concatenate([Weg1[:, HID:], Weu1], axis=1)   # [8, 64]
    bcat0 = np.concatenate([np.asarray(bg0, np.float32)[HID:], np.asarray(bu0, np.float32)])
    bcat1 = np.concatenate([np.asarray(bg1, np.float32)[HID:], np.asarray(bu1, np.float32)])
    pb0 = agg_ea @ Wecat0 + bcat0[None, :]                   # [N_PAD, 64]
    pb1 = agg_ea @ Wecat1 + bcat1[None, :]
    for c in range(NCORES):
        s = c * NODES_PER_CORE
        # [128 off, NBLK, 64]
        per_core[c]['pb0'] = np.ascontiguousarray(
            pb0[s:s + NODES_PER_CORE].reshape(NBLK, 128, 64).transpose(1, 0, 2))
        per_core[c]['pb1'] = np.ascontiguousarray(
            pb1[s:s + NODES_PER_CORE].reshape(NBLK, 128, 64).transpose(1, 0, 2))

    # --- weights
    Wcat0 = np.concatenate([Wg0[:CIN, HID:], Wu0[:CIN]], axis=1)      # [16, 64]
    Wcat1 = np.concatenate([Wg1[:HID, HID:], Wu1[:HID]], axis=1)      # [32, 64]
    w0_bd = np.zeros((96, 384), np.float32)      # 6 t-blocks of [16, 64]
    for tt in range(6):
        w0_bd[tt * 16:(tt + 1) * 16, tt * 64:(tt + 1) * 64] = Wcat0
    w1_bd = np.zeros((96, 192), np.float32)      # 3 t-blocks of [32, 64]
    for tt in range(3):
        w1_bd[tt * 32:(tt + 1) * 32, tt * 64:(tt + 1) * 64] = Wcat1

    ident = _to_bf16(np.eye(128, dtype=np.float32))
    shared = dict(wcat0_rep=_to_bf16(w0_bd), wcat1_rep=_to_bf16(w1_bd),
                  ident=ident)
    gates_key = tuple(tuple(int(g) for g in row) for row in gates)
    bnd_key = tuple(bnd)
    return shared, per_core, perm_slot, C, gates_key, bnd_key


# ---------------------------------------------------------------- bass build
import os


def _build_nc(C, gates, bnd):
    E_blk = C * 128
    E_pad = NBLK * E_blk
    nsub = len(bnd) - 1
    nc = bacc.Bacc(None, target_bir_lowering=False, num_swdge_queues=4)

    xe_d = nc.declare_dram_parameter("xe", [128, NBLK * C, T * CIN], DT.bfloat16, isOutput=False)
    idx_d = nc.declare_dram_parameter("idx", [128, E_pad // 16], DT.int16, isOutput=False)
    sc_d = nc.declare_dram_parameter("sc", [128, NBLK * C, 128], DT.bfloat16, isOutput=False)
    w0_d = nc.declare_dram_parameter("wcat0_rep", [96, 384], DT.bfloat16, isOutput=False)
    w1_d = nc.declare_dram_parameter("wcat1_rep", [96, 192], DT.bfloat16, isOutput=False)
    pb0_d = nc.declare_dram_parameter("pb0", [128, NBLK, 64], DT.float32, isOutput=False)
    pb1_d = nc.declare_dram_parameter("pb1", [128, NBLK, 64], DT.float32, isOutput=False)
    ident_d = nc.declare_dram_parameter("ident", [128, 128], DT.bfloat16, isOutput=False)
    out_d = nc.declare_dram_parameter("out", [NODES_PER_CORE, T * HID], DT.float32, isOutput=True)

    h_table = nc.dram_tensor("h_table", [N_PAD, T * HID], DT.bfloat16)
    NG = NBLK // GB
    h_slices = [nc.dram_tensor(f"h_slice{g}", [GB * 128, T * HID], DT.bfloat16)
                for g in range(NG)]
    h_tmps = [nc.dram_tensor(f"h_tmp{g}", [GB * NCORES * 128, T * HID],
                             DT.bfloat16, addr_space="Shared")
              for g in range(NG)]

    with tile.TileContext(nc) as tc:
        with (
            tc.tile_pool(name="const", bufs=1) as constp,
            tc.tile_pool(name="big", bufs=1) as bigp,
            tc.tile_pool(name="msg1", bufs=NBLK) as msg1p,
            tc.tile_pool(name="work", bufs=2) as workp,
            tc.tile_pool(name="psum_cat", bufs=2, space="PSUM") as pcatp,
            tc.tile_pool(name="psum_mix", bufs=2, space="PSUM") as pmixp,
            tc.tile_pool(name="psum_xw", bufs=1, space="PSUM") as pxwp,
        ):
            ident_sb = constp.tile([128, 128], DT.bfloat16)
            nc.sync.dma_start(ident_sb[:], ident_d[:])

            # ---- constants / streamed inputs
            xe_sb = bigp.tile([128, NBLK * C, T * CIN], DT.bfloat16)
            sc_sb = bigp.tile([128, NBLK * C, 128], DT.bfloat16)
            for b in range(NBLK):
                nc.sync.dma_start(xe_sb[:, b * C:(b + 1) * C, :],
                                  xe_d[:, b * C:(b + 1) * C, :])
                nc.sync.dma_start(sc_sb[:, b * C:(b + 1) * C, :],
                                  sc_d[:, b * C:(b + 1) * C, :])
            idx_sb = constp.tile([128, E_pad // 16], DT.int16)
            nc.sync.dma_start(idx_sb[:], idx_d[:])
            w0_sb = constp.tile([96, 384], DT.bfloat16)
            nc.sync.dma_start(w0_sb[:], w0_d[:])
            w1_sb = constp.tile([96, 192], DT.bfloat16)
            nc.sync.dma_start(w1_sb[:], w1_d[:])
            pb0_sb = constp.tile([128, NBLK, 64], DT.float32)
            nc.sync.dma_start(pb0_sb[:], pb0_d[:])
            pb1_sb = constp.tile([128, NBLK, 64], DT.float32)
            nc.sync.dma_start(pb1_sb[:], pb1_d[:])

            gq = [0]

            def compute_block(lidx, b, rhs_tile, rhs_off):
                cw = CIN if lidx == 0 else HID        # channels per t
                fa = T * cw                           # agg width (192 / 384)
                nhalf = fa // 96                      # transpose halves (2 / 4)
                pcat = pcatp.tile([128, fa], DT.float32, tag="pcat")
                for ch in range(C):
                    nc.tensor.matmul(pcat[:], sc_sb[:, b * C + ch, :],
                                     rhs_tile[:, rhs_off + ch, 0:fa],
                                     start=(ch == 0), stop=(ch == C - 1))
                pb_sb = pb0_sb if lidx == 0 else pb1_sb
                agg_bf = workp.tile([128, fa], DT.bfloat16, tag="agg_bf")
                nc.vector.tensor_copy(agg_bf[:], pcat[:])
                aggT = workp.tile([96, nhalf, 128], DT.bfloat16, tag="aggT")
                for hh in range(nhalf):
                    pT = pmixp.tile([96, 128], DT.bfloat16, tag="pmix")
                    nc.tensor.transpose(pT[:], agg_bf[:, hh * 96:(hh + 1) * 96],
                                        ident_sb[:])
                    nc.vector.tensor_copy(aggT[:, hh, :], pT[:])
                pxw = pxwp.tile([128, 1024], DT.float32, tag="pxw")
                if lidx == 0:
                    nh, hstride, width, na, wsb = 2, 512, 384, 6, w0_sb
                else:
                    nh, hstride, width, na, wsb = 4, 256, 192, 3, w1_sb
                for hh in range(nh):
                    nc.tensor.matmul(
                        pxw[:, hh * hstride:hh * hstride + width],
                        aggT[:, hh, :], wsb[:], start=True, stop=True)
                xw_view = (pxw[:].rearrange("p (h x) -> p h x", h=nh)
                           [:, :, 0:width]
                           .rearrange("p h (a d) -> p h a d", d=64))
                pre = workp.tile([128, T, 64], DT.float32, tag="pre")
                nc.vector.tensor_add(
                    pre[:].rearrange("p (h a) d -> p h a d", h=nh),
                    xw_view,
                    pb_sb[:, b, :].unsqueeze(1).unsqueeze(1)
                    .broadcast_to((128, nh, na, 64)))
                oneR = workp.tile([128, T, 32], DT.float32, tag="oneR")
                nc.scalar.activation(oneR[:], pre[:, :, 0:32],
                                     mybir.ActivationFunctionType.Sigmoid,
                                     scale=-1.0)
                hc = workp.tile([128, T, 32], DT.float32, tag="hc")
                nc.scalar.activation(hc[:], pre[:, :, 32:64],
                                     mybir.ActivationFunctionType.Tanh)
                if lidx == 0:
                    # h = (1-R) * relu(HC), bf16 (raw; norms live in sc)
                    h_bf = workp.tile([128, T * HID], DT.bfloat16, tag="h_bf")
                    nc.vector.scalar_tensor_tensor(
                        h_bf[:].rearrange("p (t d) -> p t d", d=32),
                        hc[:], 0.0, oneR[:],
                        mybir.AluOpType.max, mybir.AluOpType.mult)
                    nc.sync.dma_start(
                        h_slices[b // GB][(b % GB) * 128:(b % GB + 1) * 128, :],
                        h_bf[:])
                else:
                    o_sb = workp.tile([128, T * HID], DT.float32, tag="o_sb")
                    nc.vector.tensor_mul(
                        o_sb[:].rearrange("p (t d) -> p t d", d=32),
                        hc[:], oneR[:])
                    nc.sync.dma_start(out_d[b * 128:(b + 1) * 128, :], o_sb[:])

            # ---- layer 0 + h exchange: per-group AllGathers into tmp Shared
            # tensors (their own input tensors, so each AG's trigger only
            # waits on its GB blocks), then plain DRAM copies into h_table
            # (range-tracked, so gate-0 gathers don't wait on later groups)
            grows = GB * NCORES * 128
            for b in range(NBLK):
                compute_block(0, b, xe_sb, b * C)
                if (b + 1) % GB == 0:
                    g = b // GB
                    nc.gpsimd.collective_compute(
                        "AllGather", mybir.AluOpType.bypass,
                        replica_groups=[list(range(NCORES))],
                        ins=[h_slices[g][:]], outs=[h_tmps[g][:]])
                    nc.sync.dma_start(
                        h_table[g * grows:(g + 1) * grows, :], h_tmps[g][:])

            # ---- layer-1 gathers: (gate, block, subrange) sorted by gate.
            # Issued after all AG triggers; each descgen waits only on the
            # AllGather chunks covering its source blocks.
            msgs1 = {}
            for b in range(NBLK):
                msgs1[b] = msg1p.tile([128, C, T * HID], DT.bfloat16,
                                      tag="m1", name=f"msg1_{b}")
            order = sorted((gates[b][s], b, s)
                           for b in range(NBLK) for s in range(nsub))
            rows_per_gate = GB * NCORES * 128
            for g, b, s in order:
                lo, hi = bnd[s], bnd[s + 1]
                nrow = (hi - lo) * 128
                # prefix-slice the source so the dep tracker only gates this
                # sub-gather on the AllGather groups 0..g it actually reads
                src_ap = h_table[0:(g + 1) * rows_per_gate, :]
                nc.gpsimd.dma_gather(
                    msgs1[b][:, lo:hi, :], src_ap,
                    idx_sb[:, (b * E_blk + lo * 128) // 16:(b * E_blk + hi * 128) // 16],
                    nrow, nrow, T * HID, single_packet=True,
                    queue_num=gq[0] % 4)
                gq[0] += 1

            # ---- layer 1
            for b in range(NBLK):
                compute_block(1, b, msgs1[b], 0)

    nc.compile()
    return nc


_NC_CACHE = {}
_LAST_RESULT = None


def kernel(**inputs) -> np.ndarray:
    shared, per_core, perm_slot, C, gates, bnd = _host_prep(**inputs)
    key = (C, gates, bnd)
    if key not in _NC_CACHE:
        _NC_CACHE[key] = _build_nc(C, gates, bnd)
    nc = _NC_CACHE[key]
    in_maps = []
    for c in range(NCORES):
        m = dict(
            xe=np.ascontiguousarray(per_core[c]['xe']),
            idx=np.ascontiguousarray(per_core[c]['idx']),
            sc=np.ascontiguousarray(per_core[c]['sc']),
            pb0=per_core[c]['pb0'], pb1=per_core[c]['pb1'],
            wcat0_rep=shared['wcat0_rep'], wcat1_rep=shared['wcat1_rep'],
            ident=shared['ident'],
        )
        in_maps.append(m)
    trace = bool(os.environ.get('KTRACE'))
    if trace:
        try:
            import ntff_shim  # registers the axon NTFF profile hook
        except Exception:
            pass
    res = run_bass_kernel_spmd(nc, in_maps, core_ids=list(range(NCORES)),
                               trace=trace)
    global _LAST_RESULT
    _LAST_RESULT = res
    out_pad = np.concatenate([res.results[c]["out"] for c in range(NCORES)], axis=0)
    out = out_pad[perm_slot].reshape(N, T, HID).transpose(1, 0, 2)
    return np.ascontiguousarray(out.astype(np.float32))


if __name__ == "__main__":
    pass


# revision 27
# speedup vs baseline: 1.1035x; 1.1035x over previous
"""Trainium2 Bass kernel for nn_AGCRN_Model (gnn_message_passing).

Self-contained: call kernel(**inputs) with the full reference inputs.

Algorithm (reference simplifies because H==0 throughout):
  per layer L: pre = A_norm @ x @ Wcat_L + A_norm-agg(eattr @ Wecat_L) + b_L
               h   = (1-sigmoid(pre_R)) * tanh(pre_U)   [relu after layer 0]
Sharding: by destination node. Core c owns 1280 node slots (10 blocks x 128),
all 12 timesteps. Host relabels nodes (degree-balanced blocks) and bakes the
GCN norm rs[dst]*rs[src] into one-hot scatter matrices sc.

v2 design (vs baseline):
- Layer 0 does NO device gather: the host pre-stages x rows in edge order
  (xe table, [128, NBLK*C, 192] bf16) so the device streams it contiguously.
- The edge-feature term (sum_e norm*ea @ Wecat + bias) is precomputed on the
  host into per-block prebase tables (T-independent static graph prep).
- h exchange is a pipelined sequence of per-block AllGathers into a
  block-major h_table (row = src_blk*1024 + src_core*128 + off), each issued
  right after its L0 block finishes; a tiny warmup collective at t~0 absorbs
  the cc bootstrap barrier.
- Layer-1 gathers are split into 3 chunk-aligned sub-gathers per block,
  edges sorted by source block so each sub-gather only needs the first few
  AllGather chunks (gates precomputed on host, maxed across cores so one
  NEFF serves all 8 cores).
"""
import sys

sys.path.insert(0, '/opt/trn_rl_repo')

import numpy as np
import ml_dtypes

import concourse.bass as bass
import concourse.mybir as mybir
from concourse import bacc, tile
from concourse.bass_utils import run_bass_kernel_spmd

N = 10000
E = 100000
T = 12
CIN = 16
CE = 8
HID = 32
NCORES = 8
NBLK = 10
NODES_PER_CORE = NBLK * 128          # 1280
N_PAD = NCORES * NODES_PER_CORE      # 10240
NSUB = 2                             # L1 sub-gathers per block
EXCHANGE = "ag2"                     # "ag1" | "ag2" (two grouped AllGathers)
GB = 5                               # L0 blocks per AllGather group (ag2)

bf16 = ml_dtypes.bfloat16
DT = mybir.dt


def _to_bf16(a):
    return np.asarray(a).astype(bf16)


# ---------------------------------------------------------------- host prep
def _host_prep(x, edge_index, edge_attr,
               Wg0, Weg0, bg0, Wu0, Weu0, bu0,
               Wg1, Weg1, bg1, Wu1, Weu1, bu1):
    X = np.asarray(x, np.float32)[0]                  # [T, N, CIN]
    src = np.asarray(edge_index[0]).astype(np.int64)
    dst = np.asarray(edge_index[1]).astype(np.int64)
    ea = np.asarray(edge_attr, np.float32)            # [E, CE]

    deg = np.maximum(np.bincount(dst, minlength=N).astype(np.float32), 1.0)
    rs = 1.0 / np.sqrt(deg)

    # --- node -> (core, block, offset): snake-deal by degree, then bin-pack
    order = np.argsort(-deg, kind='stable')
    core_of = np.empty(N, np.int64)
    for i, n in enumerate(order):
        k = i % (2 * NCORES)
        core_of[n] = k if k < NCORES else 2 * NCORES - 1 - k
    perm_slot = np.empty(N, np.int64)
    for c in range(NCORES):
        nodes_c = np.where(core_of == c)[0]
        nodes_c = nodes_c[np.argsort(-deg[nodes_c], kind='stable')]
        blk_load = np.zeros(NBLK)
        blk_fill = np.zeros(NBLK, np.int64)
        for n in nodes_c:
            cand = np.where(blk_fill < 128)[0]
            b = cand[np.argmin(blk_load[cand])]
            perm_slot[n] = c * NODES_PER_CORE + b * 128 + blk_fill[b]
            blk_fill[b] += 1
            blk_load[b] += deg[n]

    # x rows keyed by node, t-major: [N, T*CIN]
    xrow = np.ascontiguousarray(X.transpose(1, 0, 2).reshape(N, T * CIN))

    # --- edge -> dst slot / src h_table row (block-major table layout)
    dslot = perm_slot[dst]
    dcore = dslot // NODES_PER_CORE
    dblk = (dslot % NODES_PER_CORE) // 128
    doff = dslot % 128
    sslot = perm_slot[src]
    s_core = sslot // NODES_PER_CORE
    s_blk = (sslot % NODES_PER_CORE) // 128
    s_off = sslot % 128
    # group-rank-major: AllGather of blocks [g*GB,(g+1)*GB) lands rows
    # [g*GB*1024, ...) in (core, local block, off) order (GB=NBLK -> one AG,
    # plain rank-major)
    s_gate = s_blk // GB
    s_row = (s_gate * (NCORES * GB * 128) + s_core * (GB * 128)
             + (s_blk % GB) * 128 + s_off)
    enorm = rs[dst] * rs[src]

    maxblk = 0
    core_edges = []
    for c in range(NCORES):
        per_blk = []
        for b in range(NBLK):
            es = np.where((dcore == c) & (dblk == b))[0]
            # sort by (AG group, src row) so early sub-gathers only need the
            # first AllGather chunks, and reads are ~sequential in h_table
            es = es[np.lexsort((s_row[es], s_gate[es]))]
            per_blk.append(es)
            maxblk = max(maxblk, len(es))
        core_edges.append(per_blk)
    C = int(np.ceil(maxblk / 128))
    E_blk = C * 128
    E_pad = NBLK * E_blk

    # chunk-range boundaries for the NSUB sub-gathers (shared across cores);
    # first boundary below half so wave 1 usually needs only AG group 0
    if NSUB == 2:
        bnd = [0, max(1, int(np.floor(C * 0.4))), C]
    else:
        bnd = [0] + [int(np.ceil(C * (k + 1) / NSUB)) for k in range(NSUB)]
    bnd = sorted(set(bnd))  # strictly increasing, ends at C
    gates = np.zeros((NBLK, len(bnd) - 1), np.int64)  # min AG chunk needed

    per_core = []
    for c in range(NCORES):
        idx = np.zeros(E_pad, np.int16)
        grp = np.zeros(E_pad, np.int64)
        sc = np.zeros((NBLK * C, 128, 128), np.float32)   # (chunk, e_row, doff)
        xe = np.zeros((128, NBLK * C, T * CIN), np.float32)  # (e_row, chunk, :)
        for b in range(NBLK):
            es = core_edges[c][b]
            k = len(es)
            idx[b * E_blk:b * E_blk + k] = s_row[es].astype(np.int16)
            grp[b * E_blk:b * E_blk + k] = s_gate[es]
            rows = np.arange(k)
            sc[b * C + rows // 128, rows % 128, doff[es]] = enorm[es]
            xe[rows % 128, b * C + rows // 128, :] = xrow[src[es]]
            for s in range(len(bnd) - 1):
                lo, hi = b * E_blk + bnd[s] * 128, b * E_blk + bnd[s + 1] * 128
                # pad entries have grp 0 (they gather h_table row 0)
                gates[b, s] = max(gates[b, s], int(grp[lo:hi].max()))
        idx_w = np.tile(idx.reshape(-1, 16).T, (8, 1)).copy()
        sc_dev = _to_bf16(np.transpose(sc, (1, 0, 2)))    # [128, NBLK*C, 128]
        per_core.append(dict(idx=idx_w, sc=sc_dev, xe=_to_bf16(xe)))

    # --- per-slot prebase (edge-feature aggregate + bias), both layers
    Wg0, Wu0 = np.asarray(Wg0, np.float32), np.asarray(Wu0, np.float32)
    Wg1, Wu1 = np.asarray(Wg1, np.float32), np.asarray(Wu1, np.float32)
    Weg0, Weu0 = np.asarray(Weg0, np.float32), np.asarray(Weu0, np.float32)
    Weg1, Weu1 = np.asarray(Weg1, np.float32), np.asarray(Weu1, np.float32)
    agg_ea = np.zeros((N_PAD, CE), np.float32)
    np.add.at(agg_ea, dslot, ea * enorm[:, None])
    Wecat0 = np.concatenate([Weg0[:, HID:], Weu0], axis=1)   # [8, 64]
    Wecat1 = np.concatenate([Weg1[:, HID:], Weu1], axis=1)   # [8, 64]
    bcat0 = np.concatenate([np.asarray(bg0, np.float32)[HID:], np.asarray(bu0, np.float32)])
    bcat1 = np.concatenate([np.asarray(bg1, np.float32)[HID:], np.asarray(bu1, np.float32)])
    pb0 = agg_ea @ Wecat0 + bcat0[None, :]                   # [N_PAD, 64]
    pb1 = agg_ea @ Wecat1 + bcat1[None, :]
    for c in range(NCORES):
        s = c * NODES_PER_CORE
        # [128 off, NBLK, 64]
        per_core[c]['pb0'] = np.ascontiguousarray(
            pb0[s:s + NODES_PER_CORE].reshape(NBLK, 128, 64).transpose(1, 0, 2))
        per_core[c]['pb1'] = np.ascontiguousarray(
            pb1[s:s + NODES_PER_CORE].reshape(NBLK, 128, 64).transpose(1, 0, 2))

    # --- weights
    Wcat0 = np.concatenate([Wg0[:CIN, HID:], Wu0[:CIN]], axis=1)      # [16, 64]
    Wcat1 = np.concatenate([Wg1[:HID, HID:], Wu1[:HID]], axis=1)      # [32, 64]
    w0_bd = np.zeros((96, 384), np.float32)      # 6 t-blocks of [16, 64]
    for tt in range(6):
        w0_bd[tt * 16:(tt + 1) * 16, tt * 64:(tt + 1) * 64] = Wcat0
    w1_bd = np.zeros((96, 192), np.float32)      # 3 t-blocks of [32, 64]
    for tt in range(3):
        w1_bd[tt * 32:(tt + 1) * 32, tt * 64:(tt + 1) * 64] = Wcat1

    ident = _to_bf16(np.eye(128, dtype=np.float32))
    shared = dict(wcat0_rep=_to_bf16(w0_bd), wcat1_rep=_to_bf16(w1_bd),
                  ident=ident)
    gates_key = tuple(tuple(int(g) for g in row) for row in gates)
    bnd_key = tuple(bnd)
    return shared, per_core, perm_slot, C, gates_key, bnd_key


# ---------------------------------------------------------------- bass build
import os


def _build_nc(C, gates, bnd):
    E_blk = C * 128
    E_pad = NBLK * E_blk
    nsub = len(bnd) - 1
    nc = bacc.Bacc(None, target_bir_lowering=False, num_swdge_queues=4)

    xe_d = nc.declare_dram_parameter("xe", [128, NBLK * C, T * CIN], DT.bfloat16, isOutput=False)
    idx_d = nc.declare_dram_parameter("idx", [128, E_pad // 16], DT.int16, isOutput=False)
    sc_d = nc.declare_dram_parameter("sc", [128, NBLK * C, 128], DT.bfloat16, isOutput=False)
    w0_d = nc.declare_dram_parameter("wcat0_rep", [96, 384], DT.bfloat16, isOutput=False)
    w1_d = nc.declare_dram_parameter("wcat1_rep", [96, 192], DT.bfloat16, isOutput=False)
    pb0_d = nc.declare_dram_parameter("pb0", [128, NBLK, 64], DT.float32, isOutput=False)
    pb1_d = nc.declare_dram_parameter("pb1", [128, NBLK, 64], DT.float32, isOutput=False)
    ident_d = nc.declare_dram_parameter("ident", [128, 128], DT.bfloat16, isOutput=False)
    out_d = nc.declare_dram_parameter("out", [NODES_PER_CORE, T * HID], DT.float32, isOutput=True)

    h_table = nc.dram_tensor("h_table", [N_PAD, T * HID], DT.bfloat16,
                             addr_space="Shared")
    NG = NBLK // GB
    h_slices = [nc.dram_tensor(f"h_slice{g}", [GB * 128, T * HID], DT.bfloat16)
                for g in range(NG)]

    with tile.TileContext(nc) as tc:
        with (
            tc.tile_pool(name="const", bufs=1) as constp,
            tc.tile_pool(name="big", bufs=1) as bigp,
            tc.tile_pool(name="msg1", bufs=NBLK) as msg1p,
            tc.tile_pool(name="work", bufs=2) as workp,
            tc.tile_pool(name="psum_cat", bufs=2, space="PSUM") as pcatp,
            tc.tile_pool(name="psum_mix", bufs=2, space="PSUM") as pmixp,
            tc.tile_pool(name="psum_xw", bufs=1, space="PSUM") as pxwp,
        ):
            ident_sb = constp.tile([128, 128], DT.bfloat16)
            nc.sync.dma_start(ident_sb[:], ident_d[:])

            # ---- constants / streamed inputs
            xe_sb = bigp.tile([128, NBLK * C, T * CIN], DT.bfloat16)
            sc_sb = bigp.tile([128, NBLK * C, 128], DT.bfloat16)
            for b in range(NBLK):
                nc.sync.dma_start(xe_sb[:, b * C:(b + 1) * C, :],
                                  xe_d[:, b * C:(b + 1) * C, :])
                nc.sync.dma_start(sc_sb[:, b * C:(b + 1) * C, :],
                                  sc_d[:, b * C:(b + 1) * C, :])
            idx_sb = constp.tile([128, E_pad // 16], DT.int16)
            nc.sync.dma_start(idx_sb[:], idx_d[:])
            w0_sb = constp.tile([96, 384], DT.bfloat16)
            nc.sync.dma_start(w0_sb[:], w0_d[:])
            w1_sb = constp.tile([96, 192], DT.bfloat16)
            nc.sync.dma_start(w1_sb[:], w1_d[:])
            pb0_sb = constp.tile([128, NBLK, 64], DT.float32)
            nc.sync.dma_start(pb0_sb[:], pb0_d[:])
            pb1_sb = constp.tile([128, NBLK, 64], DT.float32)
            nc.sync.dma_start(pb1_sb[:], pb1_d[:])

            gq = [0]

            def compute_block(lidx, b, rhs_tile, rhs_off):
                cw = CIN if lidx == 0 else HID        # channels per t
                fa = T * cw                           # agg width (192 / 384)
                nhalf = fa // 96                      # transpose halves (2 / 4)
                pcat = pcatp.tile([128, fa], DT.float32, tag="pcat")
                for ch in range(C):
                    nc.tensor.matmul(pcat[:], sc_sb[:, b * C + ch, :],
                                     rhs_tile[:, rhs_off + ch, 0:fa],
                                     start=(ch == 0), stop=(ch == C - 1))
                pb_sb = pb0_sb if lidx == 0 else pb1_sb
                agg_bf = workp.tile([128, fa], DT.bfloat16, tag="agg_bf")
                nc.vector.tensor_copy(agg_bf[:], pcat[:])
                aggT = workp.tile([96, nhalf, 128], DT.bfloat16, tag="aggT")
                for hh in range(nhalf):
                    pT = pmixp.tile([96, 128], DT.bfloat16, tag="pmix")
                    nc.tensor.transpose(pT[:], agg_bf[:, hh * 96:(hh + 1) * 96],
                                        ident_sb[:])
                    nc.vector.tensor_copy(aggT[:, hh, :], pT[:])
                pxw = pxwp.tile([128, 1024], DT.float32, tag="pxw")
                if lidx == 0:
                    nh, hstride, width, na, wsb = 2, 512, 384, 6, w0_sb
                else:
                    nh, hstride, width, na, wsb = 4, 256, 192, 3, w1_sb
                for hh in range(nh):
                    nc.tensor.matmul(
                        pxw[:, hh * hstride:hh * hstride + width],
                        aggT[:, hh, :], wsb[:], start=True, stop=True)
                xw_view = (pxw[:].rearrange("p (h x) -> p h x", h=nh)
                           [:, :, 0:width]
                           .rearrange("p h (a d) -> p h a d", d=64))
                pre = workp.tile([128, T, 64], DT.float32, tag="pre")
                nc.vector.tensor_add(
                    pre[:].rearrange("p (h a) d -> p h a d", h=nh),
                    xw_view,
                    pb_sb[:, b, :].unsqueeze(1).unsqueeze(1)
                    .broadcast_to((128, nh, na, 64)))
                oneR = workp.tile([128, T, 32], DT.float32, tag="oneR")
                nc.scalar.activation(oneR[:], pre[:, :, 0:32],
                                     mybir.ActivationFunctionType.Sigmoid,
                                     scale=-1.0)
                hc = workp.tile([128, T, 32], DT.float32, tag="hc")
                nc.scalar.activation(hc[:], pre[:, :, 32:64],
                                     mybir.ActivationFunctionType.Tanh)
                if lidx == 0:
                    # h = (1-R) * relu(HC), bf16 (raw; norms live in sc)
                    h_bf = workp.tile([128, T * HID], DT.bfloat16, tag="h_bf")
                    nc.vector.scalar_tensor_tensor(
                        h_bf[:].rearrange("p (t d) -> p t d", d=32),
                        hc[:], 0.0, oneR[:],
                        mybir.AluOpType.max, mybir.AluOpType.mult)
                    nc.sync.dma_start(
                        h_slices[b // GB][(b % GB) * 128:(b % GB + 1) * 128, :],
                        h_bf[:])
                else:
                    o_sb = workp.tile([128, T * HID], DT.float32, tag="o_sb")
                    nc.vector.tensor_mul(
                        o_sb[:].rearrange("p (t d) -> p t d", d=32),
                        hc[:], oneR[:])
                    nc.sync.dma_start(out_d[b * 128:(b + 1) * 128, :], o_sb[:])

            # ---- layer 0 + h exchange: per-group AllGathers (each group has
            # its own input tensor so its trigger only waits on its GB blocks)
            grows = GB * NCORES * 128
            for b in range(NBLK):
                compute_block(0, b, xe_sb, b * C)
                if (b + 1) % GB == 0:
                    g = b // GB
                    nc.gpsimd.collective_compute(
                        "AllGather", mybir.AluOpType.bypass,
                        replica_groups=[list(range(NCORES))],
                        ins=[h_slices[g][:]],
                        outs=[h_table[g * grows:(g + 1) * grows, :]])

            # ---- layer-1 gathers: (gate, block, subrange) sorted by gate.
            # Issued after all AG triggers; each descgen waits only on the
            # AllGather chunks covering its source blocks.
            msgs1 = {}
            for b in range(NBLK):
                msgs1[b] = msg1p.tile([128, C, T * HID], DT.bfloat16,
                                      tag="m1", name=f"msg1_{b}")
            order = sorted((gates[b][s], b, s)
                           for b in range(NBLK) for s in range(nsub))
            rows_per_gate = GB * NCORES * 128
            for g, b, s in order:
                lo, hi = bnd[s], bnd[s + 1]
                nrow = (hi - lo) * 128
                # prefix-slice the source so the dep tracker only gates this
                # sub-gather on the AllGather groups 0..g it actually reads
                src_ap = h_table[0:(g + 1) * rows_per_gate, :]
                nc.gpsimd.dma_gather(
                    msgs1[b][:, lo:hi, :], src_ap,
                    idx_sb[:, (b * E_blk + lo * 128) // 16:(b * E_blk + hi * 128) // 16],
                    nrow, nrow, T * HID, single_packet=True,
                    queue_num=gq[0] % 4)
                gq[0] += 1

            # ---- layer 1
            for b in range(NBLK):
                compute_block(1, b, msgs1[b], 0)

    nc.compile()
    return nc


_NC_CACHE = {}
_LAST_RESULT = None


def kernel(**inputs) -> np.ndarray:
    shared, per_core, perm_slot, C, gates, bnd = _host_prep(**inputs)
    key = (C, gates, bnd)
    if key not in _NC_CACHE:
        _NC_CACHE[key] = _build_nc(C, gates, bnd)
    nc = _NC_CACHE[key]
    in_maps = []
    for c in range(NCORES):
        m = dict(
            xe=np.ascontiguousarray(per_core[c]['xe']),
            idx=np.ascontiguousarray(per_core[c]['idx']),
            sc=np.ascontiguousarray(per_core[c]['sc']),
            pb0=per_core[c]['pb0'], pb1=per_core[c]['pb1'],
            wcat0_rep=shared['wcat0_rep'], wcat1_rep=shared['wcat1_rep'],
            ident=shared['ident'],
        )
        in_maps.append(m)
    trace = bool(os.environ.get('KTRACE'))
    if trace:
        try:
            import ntff_shim  # registers the axon NTFF profile hook
        except Exception:
            pass
    res = run_bass_kernel_spmd(nc, in_maps, core_ids=list(range(NCORES)),
                               trace=trace)
    global _LAST_RESULT
    _LAST_RESULT = res
    out_pad = np.concatenate([res.results[c]["out"] for c in range(NCORES)], axis=0)
    out = out_pad[perm_slot].reshape(N, T, HID).transpose(1, 0, 2)
    return np.ascontiguousarray(out.astype(np.float32))


if __name__ == "__main__":
    pass


# revision 34
# speedup vs baseline: 1.2580x; 1.1400x over previous
"""Trainium2 Bass kernel for nn_AGCRN_Model (gnn_message_passing).

Self-contained: call kernel(**inputs) with the full reference inputs.

Algorithm (reference simplifies because H==0 throughout):
  per layer L: pre = A_norm @ x @ Wcat_L + A_norm-agg(eattr @ Wecat_L) + b_L
               h   = (1-sigmoid(pre_R)) * tanh(pre_U)   [relu after layer 0]
Sharding: by destination node. Core c owns 1280 node slots (10 blocks x 128),
all 12 timesteps. Host relabels nodes (degree-balanced blocks) and bakes the
GCN norm rs[dst]*rs[src] into one-hot scatter matrices sc.

v2 design (vs baseline):
- Layer 0 does NO device gather: the host pre-stages x rows in edge order
  (xe table, [128, NBLK*C, 192] bf16) so the device streams it contiguously.
- The edge-feature term (sum_e norm*ea @ Wecat + bias) is precomputed on the
  host into per-block prebase tables (T-independent static graph prep).
- h exchange is a pipelined sequence of per-block AllGathers into a
  block-major h_table (row = src_blk*1024 + src_core*128 + off), each issued
  right after its L0 block finishes; a tiny warmup collective at t~0 absorbs
  the cc bootstrap barrier.
- Layer-1 gathers are split into 3 chunk-aligned sub-gathers per block,
  edges sorted by source block so each sub-gather only needs the first few
  AllGather chunks (gates precomputed on host, maxed across cores so one
  NEFF serves all 8 cores).
"""
import sys

sys.path.insert(0, '/opt/trn_rl_repo')

import numpy as np
import ml_dtypes

import concourse.bass as bass
import concourse.mybir as mybir
from concourse import bacc, tile
from concourse.bass_utils import run_bass_kernel_spmd

N = 10000
E = 100000
T = 12
CIN = 16
CE = 8
HID = 32
NCORES = 8
NBLK = 10
NODES_PER_CORE = NBLK * 128          # 1280
N_PAD = NCORES * NODES_PER_CORE      # 10240
NSUB = 2                             # L1 sub-gathers per block
GROUPS = [7, 3]                      # L0 blocks per AllGather group
GSTART = [0, 7]
SPLIT_FRAC = 0.65                    # chunk fraction in gather wave 1

bf16 = ml_dtypes.bfloat16
DT = mybir.dt


def _to_bf16(a):
    return np.asarray(a).astype(bf16)


# ---------------------------------------------------------------- host prep
def _host_prep(x, edge_index, edge_attr,
               Wg0, Weg0, bg0, Wu0, Weu0, bu0,
               Wg1, Weg1, bg1, Wu1, Weu1, bu1):
    X = np.asarray(x, np.float32)[0]                  # [T, N, CIN]
    src = np.asarray(edge_index[0]).astype(np.int64)
    dst = np.asarray(edge_index[1]).astype(np.int64)
    ea = np.asarray(edge_attr, np.float32)            # [E, CE]

    deg = np.maximum(np.bincount(dst, minlength=N).astype(np.float32), 1.0)
    rs = 1.0 / np.sqrt(deg)

    # --- node -> (core, block, offset): snake-deal by degree, then bin-pack
    order = np.argsort(-deg, kind='stable')
    core_of = np.empty(N, np.int64)
    for i, n in enumerate(order):
        k = i % (2 * NCORES)
        core_of[n] = k if k < NCORES else 2 * NCORES - 1 - k
    perm_slot = np.empty(N, np.int64)
    for c in range(NCORES):
        nodes_c = np.where(core_of == c)[0]
        nodes_c = nodes_c[np.argsort(-deg[nodes_c], kind='stable')]
        blk_load = np.zeros(NBLK)
        blk_fill = np.zeros(NBLK, np.int64)
        for n in nodes_c:
            cand = np.where(blk_fill < 128)[0]
            b = cand[np.argmin(blk_load[cand])]
            perm_slot[n] = c * NODES_PER_CORE + b * 128 + blk_fill[b]
            blk_fill[b] += 1
            blk_load[b] += deg[n]

    # x rows keyed by node, t-major: [N, T*CIN]
    xrow = np.ascontiguousarray(X.transpose(1, 0, 2).reshape(N, T * CIN))

    # --- edge -> dst slot / src h_table row (block-major table layout)
    dslot = perm_slot[dst]
    dcore = dslot // NODES_PER_CORE
    dblk = (dslot % NODES_PER_CORE) // 128
    doff = dslot % 128
    sslot = perm_slot[src]
    s_core = sslot // NODES_PER_CORE
    s_blk = (sslot % NODES_PER_CORE) // 128
    s_off = sslot % 128
    # group-rank-major: the AllGather of group g (blocks GSTART[g]..) lands
    # its rows at base[g] in (core, local block, off) order
    s_gate = np.zeros(E, np.int64)
    s_row = np.zeros(E, np.int64)
    base = 0
    for g, gb in enumerate(GROUPS):
        gs = GSTART[g]
        m = (s_blk >= gs) & (s_blk < gs + gb)
        s_gate[m] = g
        s_row[m] = (base + s_core[m] * (gb * 128)
                    + (s_blk[m] - gs) * 128 + s_off[m])
        base += NCORES * gb * 128
    enorm = rs[dst] * rs[src]

    maxblk = 0
    core_edges = []
    for c in range(NCORES):
        per_blk = []
        for b in range(NBLK):
            es = np.where((dcore == c) & (dblk == b))[0]
            # sort by (AG group, src row) so early sub-gathers only need the
            # first AllGather chunks, and reads are ~sequential in h_table
            es = es[np.lexsort((s_row[es], s_gate[es]))]
            per_blk.append(es)
            maxblk = max(maxblk, len(es))
        core_edges.append(per_blk)
    C = int(np.ceil(maxblk / 128))
    E_blk = C * 128
    E_pad = NBLK * E_blk

    # chunk-range boundaries for the NSUB sub-gathers (shared across cores);
    # first boundary a bit under the group-0 edge fraction so wave 1 usually
    # needs only AG group 0
    bnd = sorted(set([0, max(1, int(np.floor(C * SPLIT_FRAC))), C]))
    gates = np.zeros((NBLK, len(bnd) - 1), np.int64)  # min AG chunk needed

    per_core = []
    for c in range(NCORES):
        idx = np.zeros(E_pad, np.int16)
        grp = np.zeros(E_pad, np.int64)
        sc = np.zeros((NBLK * C, 128, 128), np.float32)   # (chunk, e_row, doff)
        xe = np.zeros((128, NBLK * C, T * CIN), np.float32)  # (e_row, chunk, :)
        for b in range(NBLK):
            es = core_edges[c][b]
            k = len(es)
            idx[b * E_blk:b * E_blk + k] = s_row[es].astype(np.int16)
            grp[b * E_blk:b * E_blk + k] = s_gate[es]
            rows = np.arange(k)
            sc[b * C + rows // 128, rows % 128, doff[es]] = enorm[es]
            xe[rows % 128, b * C + rows // 128, :] = xrow[src[es]]
            for s in range(len(bnd) - 1):
                lo, hi = b * E_blk + bnd[s] * 128, b * E_blk + bnd[s + 1] * 128
                # pad entries have grp 0 (they gather h_table row 0)
                gates[b, s] = max(gates[b, s], int(grp[lo:hi].max()))
        idx_w = np.tile(idx.reshape(-1, 16).T, (8, 1)).copy()
        sc_dev = _to_bf16(np.transpose(sc, (1, 0, 2)))    # [128, NBLK*C, 128]
        per_core.append(dict(idx=idx_w, sc=sc_dev, xe=_to_bf16(xe)))

    # --- per-slot prebase (edge-feature aggregate + bias), both layers
    Wg0, Wu0 = np.asarray(Wg0, np.float32), np.asarray(Wu0, np.float32)
    Wg1, Wu1 = np.asarray(Wg1, np.float32), np.asarray(Wu1, np.float32)
    Weg0, Weu0 = np.asarray(Weg0, np.float32), np.asarray(Weu0, np.float32)
    Weg1, Weu1 = np.asarray(Weg1, np.float32), np.asarray(Weu1, np.float32)
    agg_ea = np.zeros((N_PAD, CE), np.float32)
    np.add.at(agg_ea, dslot, ea * enorm[:, None])
    Wecat0 = np.concatenate([Weg0[:, HID:], Weu0], axis=1)   # [8, 64]
    Wecat1 = np.concatenate([Weg1[:, HID:], Weu1], axis=1)   # [8, 64]
    bcat0 = np.concatenate([np.asarray(bg0, np.float32)[HID:], np.asarray(bu0, np.float32)])
    bcat1 = np.concatenate([np.asarray(bg1, np.float32)[HID:], np.asarray(bu1, np.float32)])
    pb0 = agg_ea @ Wecat0 + bcat0[None, :]                   # [N_PAD, 64]
    pb1 = agg_ea @ Wecat1 + bcat1[None, :]
    for c in range(NCORES):
        s = c * NODES_PER_CORE
        # [128 off, NBLK, 64]
        per_core[c]['pb0'] = np.ascontiguousarray(
            pb0[s:s + NODES_PER_CORE].reshape(NBLK, 128, 64).transpose(1, 0, 2))
        per_core[c]['pb1'] = np.ascontiguousarray(
            pb1[s:s + NODES_PER_CORE].reshape(NBLK, 128, 64).transpose(1, 0, 2))

    # --- weights
    Wcat0 = np.concatenate([Wg0[:CIN, HID:], Wu0[:CIN]], axis=1)      # [16, 64]
    Wcat1 = np.concatenate([Wg1[:HID, HID:], Wu1[:HID]], axis=1)      # [32, 64]
    w0_bd = np.zeros((96, 384), np.float32)      # 6 t-blocks of [16, 64]
    for tt in range(6):
        w0_bd[tt * 16:(tt + 1) * 16, tt * 64:(tt + 1) * 64] = Wcat0
    w1_bd = np.zeros((96, 192), np.float32)      # 3 t-blocks of [32, 64]
    for tt in range(3):
        w1_bd[tt * 32:(tt + 1) * 32, tt * 64:(tt + 1) * 64] = Wcat1

    ident = _to_bf16(np.eye(128, dtype=np.float32))
    shared = dict(wcat0_rep=_to_bf16(w0_bd), wcat1_rep=_to_bf16(w1_bd),
                  ident=ident)
    gates_key = tuple(tuple(int(g) for g in row) for row in gates)
    bnd_key = tuple(bnd)
    return shared, per_core, perm_slot, C, gates_key, bnd_key


# ---------------------------------------------------------------- bass build
import os


def _build_nc(C, gates, bnd):
    E_blk = C * 128
    E_pad = NBLK * E_blk
    nsub = len(bnd) - 1
    nc = bacc.Bacc(None, target_bir_lowering=False, num_swdge_queues=4)

    xe_d = nc.declare_dram_parameter("xe", [128, NBLK * C, T * CIN], DT.bfloat16, isOutput=False)
    idx_d = nc.declare_dram_parameter("idx", [128, E_pad // 16], DT.int16, isOutput=False)
    sc_d = nc.declare_dram_parameter("sc", [128, NBLK * C, 128], DT.bfloat16, isOutput=False)
    w0_d = nc.declare_dram_parameter("wcat0_rep", [96, 384], DT.bfloat16, isOutput=False)
    w1_d = nc.declare_dram_parameter("wcat1_rep", [96, 192], DT.bfloat16, isOutput=False)
    pb0_d = nc.declare_dram_parameter("pb0", [128, NBLK, 64], DT.float32, isOutput=False)
    pb1_d = nc.declare_dram_parameter("pb1", [128, NBLK, 64], DT.float32, isOutput=False)
    ident_d = nc.declare_dram_parameter("ident", [128, 128], DT.bfloat16, isOutput=False)
    out_d = nc.declare_dram_parameter("out", [NODES_PER_CORE, T * HID], DT.float32, isOutput=True)

    h_table = nc.dram_tensor("h_table", [N_PAD, T * HID], DT.bfloat16,
                             addr_space="Shared")
    h_slices = [nc.dram_tensor(f"h_slice{g}", [gb * 128, T * HID], DT.bfloat16)
                for g, gb in enumerate(GROUPS)]

    with tile.TileContext(nc) as tc:
        with (
            tc.tile_pool(name="const", bufs=1) as constp,
            tc.tile_pool(name="big", bufs=1) as bigp,
            tc.tile_pool(name="msg1", bufs=NBLK) as msg1p,
            tc.tile_pool(name="work", bufs=2) as workp,
            tc.tile_pool(name="psum_cat", bufs=2, space="PSUM") as pcatp,
            tc.tile_pool(name="psum_mix", bufs=2, space="PSUM") as pmixp,
            tc.tile_pool(name="psum_xw", bufs=1, space="PSUM") as pxwp,
        ):
            ident_sb = constp.tile([128, 128], DT.bfloat16)
            nc.sync.dma_start(ident_sb[:], ident_d[:])

            # ---- constants / streamed inputs
            xe_sb = bigp.tile([128, NBLK * C, T * CIN], DT.bfloat16)
            sc_sb = bigp.tile([128, NBLK * C, 128], DT.bfloat16)
            for b in range(NBLK):
                nc.sync.dma_start(xe_sb[:, b * C:(b + 1) * C, :],
                                  xe_d[:, b * C:(b + 1) * C, :])
                nc.sync.dma_start(sc_sb[:, b * C:(b + 1) * C, :],
                                  sc_d[:, b * C:(b + 1) * C, :])
            idx_sb = constp.tile([128, E_pad // 16], DT.int16)
            nc.sync.dma_start(idx_sb[:], idx_d[:])
            w0_sb = constp.tile([96, 384], DT.bfloat16)
            nc.sync.dma_start(w0_sb[:], w0_d[:])
            w1_sb = constp.tile([96, 192], DT.bfloat16)
            nc.sync.dma_start(w1_sb[:], w1_d[:])
            pb0_sb = constp.tile([128, NBLK, 64], DT.float32)
            nc.sync.dma_start(pb0_sb[:], pb0_d[:])
            pb1_sb = constp.tile([128, NBLK, 64], DT.float32)
            nc.sync.dma_start(pb1_sb[:], pb1_d[:])

            gq = [0]

            def compute_block(lidx, b, rhs_tile, rhs_off):
                cw = CIN if lidx == 0 else HID        # channels per t
                fa = T * cw                           # agg width (192 / 384)
                nhalf = fa // 96                      # transpose halves (2 / 4)
                pcat = pcatp.tile([128, fa], DT.float32, tag="pcat")
                for ch in range(C):
                    nc.tensor.matmul(pcat[:], sc_sb[:, b * C + ch, :],
                                     rhs_tile[:, rhs_off + ch, 0:fa],
                                     start=(ch == 0), stop=(ch == C - 1))
                pb_sb = pb0_sb if lidx == 0 else pb1_sb
                agg_bf = workp.tile([128, fa], DT.bfloat16, tag="agg_bf")
                nc.vector.tensor_copy(agg_bf[:], pcat[:])
                aggT = workp.tile([96, nhalf, 128], DT.bfloat16, tag="aggT")
                for hh in range(nhalf):
                    pT = pmixp.tile([96, 128], DT.bfloat16, tag="pmix")
                    nc.tensor.transpose(pT[:], agg_bf[:, hh * 96:(hh + 1) * 96],
                                        ident_sb[:])
                    nc.vector.tensor_copy(aggT[:, hh, :], pT[:])
                pxw = pxwp.tile([128, 1024], DT.float32, tag="pxw")
                if lidx == 0:
                    nh, hstride, width, na, wsb = 2, 512, 384, 6, w0_sb
                else:
                    nh, hstride, width, na, wsb = 4, 256, 192, 3, w1_sb
                for hh in range(nh):
                    nc.tensor.matmul(
                        pxw[:, hh * hstride:hh * hstride + width],
                        aggT[:, hh, :], wsb[:], start=True, stop=True)
                xw_view = (pxw[:].rearrange("p (h x) -> p h x", h=nh)
                           [:, :, 0:width]
                           .rearrange("p h (a d) -> p h a d", d=64))
                pre = workp.tile([128, T, 64], DT.float32, tag="pre")
                nc.vector.tensor_add(
                    pre[:].rearrange("p (h a) d -> p h a d", h=nh),
                    xw_view,
                    pb_sb[:, b, :].unsqueeze(1).unsqueeze(1)
                    .broadcast_to((128, nh, na, 64)))
                oneR = workp.tile([128, T, 32], DT.float32, tag="oneR")
                nc.scalar.activation(oneR[:], pre[:, :, 0:32],
                                     mybir.ActivationFunctionType.Sigmoid,
                                     scale=-1.0)
                hc = workp.tile([128, T, 32], DT.float32, tag="hc")
                nc.scalar.activation(hc[:], pre[:, :, 32:64],
                                     mybir.ActivationFunctionType.Tanh)
                if lidx == 0:
                    # h = (1-R) * relu(HC), bf16 (raw; norms live in sc)
                    h_bf = workp.tile([128, T * HID], DT.bfloat16, tag="h_bf")
                    nc.vector.scalar_tensor_tensor(
                        h_bf[:].rearrange("p (t d) -> p t d", d=32),
                        hc[:], 0.0, oneR[:],
                        mybir.AluOpType.max, mybir.AluOpType.mult)
                    g = sum(1 for gs in GSTART if b >= gs) - 1
                    lb = b - GSTART[g]
                    nc.sync.dma_start(
                        h_slices[g][lb * 128:(lb + 1) * 128, :], h_bf[:])
                else:
                    o_sb = workp.tile([128, T * HID], DT.float32, tag="o_sb")
                    nc.vector.tensor_mul(
                        o_sb[:].rearrange("p (t d) -> p t d", d=32),
                        hc[:], oneR[:])
                    nc.sync.dma_start(out_d[b * 128:(b + 1) * 128, :], o_sb[:])

            # ---- layer 0 + h exchange: per-group AllGathers (each group has
            # its own input tensor so its trigger only waits on its blocks)
            gbase = [0]
            for gb in GROUPS:
                gbase.append(gbase[-1] + NCORES * gb * 128)
            for b in range(NBLK):
                compute_block(0, b, xe_sb, b * C)
                for g, gb in enumerate(GROUPS):
                    if b == GSTART[g] + gb - 1:
                        nc.gpsimd.collective_compute(
                            "AllGather", mybir.AluOpType.bypass,
                            replica_groups=[list(range(NCORES))],
                            ins=[h_slices[g][:]],
                            outs=[h_table[gbase[g]:gbase[g + 1], :]])

            # ---- layer-1 gathers: (gate, block, subrange) sorted by gate.
            # Issued after all AG triggers; each descgen waits only on the
            # AllGather chunks covering its source blocks.
            msgs1 = {}
            for b in range(NBLK):
                msgs1[b] = msg1p.tile([128, C, T * HID], DT.bfloat16,
                                      tag="m1", name=f"msg1_{b}")
            order = sorted((gates[b][s], b, s)
                           for b in range(NBLK) for s in range(nsub))
            for g, b, s in order:
                lo, hi = bnd[s], bnd[s + 1]
                nrow = (hi - lo) * 128
                # prefix-slice the source so the dep tracker only gates this
                # sub-gather on the AllGather groups 0..g it actually reads
                src_ap = h_table[0:gbase[g + 1], :]
                nc.gpsimd.dma_gather(
                    msgs1[b][:, lo:hi, :], src_ap,
                    idx_sb[:, (b * E_blk + lo * 128) // 16:(b * E_blk + hi * 128) // 16],
                    nrow, nrow, T * HID, single_packet=True,
                    queue_num=gq[0] % 4)
                gq[0] += 1

            # ---- layer 1
            for b in range(NBLK):
                compute_block(1, b, msgs1[b], 0)

    nc.compile()
    return nc


_NC_CACHE = {}
_LAST_RESULT = None


def kernel(**inputs) -> np.ndarray:
    shared, per_core, perm_slot, C, gates, bnd = _host_prep(**inputs)
    key = (C, gates, bnd)
    if key not in _NC_CACHE:
        _NC_CACHE[key] = _build_nc(C, gates, bnd)
    nc = _NC_CACHE[key]
    in_maps = []
    for c in range(NCORES):
        m = dict(
            xe=np.ascontiguousarray(per_core[c]['xe']),
            idx=np.ascontiguousarray(per_core[c]['idx']),
            sc=np.ascontiguousarray(per_core[c]['sc']),
            pb0=per_core[c]['pb0'], pb1=per_core[c]['pb1'],
            wcat0_rep=shared['wcat0_rep'], wcat1_rep=shared['wcat1_rep'],
            ident=shared['ident'],
        )
        in_maps.append(m)
    trace = bool(os.environ.get('KTRACE'))
    if trace:
        try:
            import ntff_shim  # registers the axon NTFF profile hook
        except Exception:
            pass
    res = run_bass_kernel_spmd(nc, in_maps, core_ids=list(range(NCORES)),
                               trace=trace)
    global _LAST_RESULT
    _LAST_RESULT = res
    out_pad = np.concatenate([res.results[c]["out"] for c in range(NCORES)], axis=0)
    out = out_pad[perm_slot].reshape(N, T, HID).transpose(1, 0, 2)
    return np.ascontiguousarray(out.astype(np.float32))


if __name__ == "__main__":
    pass


# revision 35
# speedup vs baseline: 1.2603x; 1.0018x over previous
"""Trainium2 Bass kernel for nn_AGCRN_Model (gnn_message_passing).

Self-contained: call kernel(**inputs) with the full reference inputs.

Algorithm (reference simplifies because H==0 throughout):
  per layer L: pre = A_norm @ x @ Wcat_L + A_norm-agg(eattr @ Wecat_L) + b_L
               h   = (1-sigmoid(pre_R)) * tanh(pre_U)   [relu after layer 0]
Sharding: by destination node. Core c owns 1280 node slots (10 blocks x 128),
all 12 timesteps. Host relabels nodes (degree-balanced blocks) and bakes the
GCN norm rs[dst]*rs[src] into one-hot scatter matrices sc.

v2 design (vs baseline):
- Layer 0 does NO device gather: the host pre-stages x rows in edge order
  (xe table, [128, NBLK*C, 192] bf16) so the device streams it contiguously.
- The edge-feature term (sum_e norm*ea @ Wecat + bias) is precomputed on the
  host into per-block prebase tables (T-independent static graph prep).
- h exchange is a pipelined sequence of per-block AllGathers into a
  block-major h_table (row = src_blk*1024 + src_core*128 + off), each issued
  right after its L0 block finishes; a tiny warmup collective at t~0 absorbs
  the cc bootstrap barrier.
- Layer-1 gathers are split into 3 chunk-aligned sub-gathers per block,
  edges sorted by source block so each sub-gather only needs the first few
  AllGather chunks (gates precomputed on host, maxed across cores so one
  NEFF serves all 8 cores).
"""
import sys

sys.path.insert(0, '/opt/trn_rl_repo')

import numpy as np
import ml_dtypes

import concourse.bass as bass
import concourse.mybir as mybir
from concourse import bacc, tile
from concourse.bass_utils import run_bass_kernel_spmd

N = 10000
E = 100000
T = 12
CIN = 16
CE = 8
HID = 32
NCORES = 8
NBLK = 10
NODES_PER_CORE = NBLK * 128          # 1280
N_PAD = NCORES * NODES_PER_CORE      # 10240
NSUB = 2                             # L1 sub-gathers per block
GROUPS = [10]                        # L0 blocks per AllGather group
GSTART = [0]
SPLIT_FRAC = 0.65                    # chunk fraction in gather wave 1

bf16 = ml_dtypes.bfloat16
DT = mybir.dt


def _to_bf16(a):
    return np.asarray(a).astype(bf16)


# ---------------------------------------------------------------- host prep
def _host_prep(x, edge_index, edge_attr,
               Wg0, Weg0, bg0, Wu0, Weu0, bu0,
               Wg1, Weg1, bg1, Wu1, Weu1, bu1):
    X = np.asarray(x, np.float32)[0]                  # [T, N, CIN]
    src = np.asarray(edge_index[0]).astype(np.int64)
    dst = np.asarray(edge_index[1]).astype(np.int64)
    ea = np.asarray(edge_attr, np.float32)            # [E, CE]

    deg = np.maximum(np.bincount(dst, minlength=N).astype(np.float32), 1.0)
    rs = 1.0 / np.sqrt(deg)

    # --- node -> (core, block, offset): snake-deal by degree, then bin-pack
    order = np.argsort(-deg, kind='stable')
    core_of = np.empty(N, np.int64)
    for i, n in enumerate(order):
        k = i % (2 * NCORES)
        core_of[n] = k if k < NCORES else 2 * NCORES - 1 - k
    perm_slot = np.empty(N, np.int64)
    for c in range(NCORES):
        nodes_c = np.where(core_of == c)[0]
        nodes_c = nodes_c[np.argsort(-deg[nodes_c], kind='stable')]
        blk_load = np.zeros(NBLK)
        blk_fill = np.zeros(NBLK, np.int64)
        for n in nodes_c:
            cand = np.where(blk_fill < 128)[0]
            b = cand[np.argmin(blk_load[cand])]
            perm_slot[n] = c * NODES_PER_CORE + b * 128 + blk_fill[b]
            blk_fill[b] += 1
            blk_load[b] += deg[n]

    # x rows keyed by node, t-major: [N, T*CIN]
    xrow = np.ascontiguousarray(X.transpose(1, 0, 2).reshape(N, T * CIN))

    # --- edge -> dst slot / src h_table row (block-major table layout)
    dslot = perm_slot[dst]
    dcore = dslot // NODES_PER_CORE
    dblk = (dslot % NODES_PER_CORE) // 128
    doff = dslot % 128
    sslot = perm_slot[src]
    s_core = sslot // NODES_PER_CORE
    s_blk = (sslot % NODES_PER_CORE) // 128
    s_off = sslot % 128
    # group-rank-major: the AllGather of group g (blocks GSTART[g]..) lands
    # its rows at base[g] in (core, local block, off) order
    s_gate = np.zeros(E, np.int64)
    s_row = np.zeros(E, np.int64)
    base = 0
    for g, gb in enumerate(GROUPS):
        gs = GSTART[g]
        m = (s_blk >= gs) & (s_blk < gs + gb)
        s_gate[m] = g
        s_row[m] = (base + s_core[m] * (gb * 128)
                    + (s_blk[m] - gs) * 128 + s_off[m])
        base += NCORES * gb * 128
    enorm = rs[dst] * rs[src]

    maxblk = 0
    core_edges = []
    for c in range(NCORES):
        per_blk = []
        for b in range(NBLK):
            es = np.where((dcore == c) & (dblk == b))[0]
            # sort by (AG group, src row) so early sub-gathers only need the
            # first AllGather chunks, and reads are ~sequential in h_table
            es = es[np.lexsort((s_row[es], s_gate[es]))]
            per_blk.append(es)
            maxblk = max(maxblk, len(es))
        core_edges.append(per_blk)
    C = int(np.ceil(maxblk / 128))
    E_blk = C * 128
    E_pad = NBLK * E_blk

    # chunk-range boundaries for the NSUB sub-gathers (shared across cores);
    # first boundary a bit under the group-0 edge fraction so wave 1 usually
    # needs only AG group 0
    bnd = sorted(set([0, max(1, int(np.floor(C * SPLIT_FRAC))), C]))
    gates = np.zeros((NBLK, len(bnd) - 1), np.int64)  # min AG chunk needed

    per_core = []
    for c in range(NCORES):
        idx = np.zeros(E_pad, np.int16)
        grp = np.zeros(E_pad, np.int64)
        sc = np.zeros((NBLK * C, 128, 128), np.float32)   # (chunk, e_row, doff)
        xe = np.zeros((128, NBLK * C, T * CIN), np.float32)  # (e_row, chunk, :)
        for b in range(NBLK):
            es = core_edges[c][b]
            k = len(es)
            idx[b * E_blk:b * E_blk + k] = s_row[es].astype(np.int16)
            grp[b * E_blk:b * E_blk + k] = s_gate[es]
            rows = np.arange(k)
            sc[b * C + rows // 128, rows % 128, doff[es]] = enorm[es]
            xe[rows % 128, b * C + rows // 128, :] = xrow[src[es]]
            for s in range(len(bnd) - 1):
                lo, hi = b * E_blk + bnd[s] * 128, b * E_blk + bnd[s + 1] * 128
                # pad entries have grp 0 (they gather h_table row 0)
                gates[b, s] = max(gates[b, s], int(grp[lo:hi].max()))
        idx_w = np.tile(idx.reshape(-1, 16).T, (8, 1)).copy()
        sc_dev = _to_bf16(np.transpose(sc, (1, 0, 2)))    # [128, NBLK*C, 128]
        per_core.append(dict(idx=idx_w, sc=sc_dev, xe=_to_bf16(xe)))

    # --- per-slot prebase (edge-feature aggregate + bias), both layers
    Wg0, Wu0 = np.asarray(Wg0, np.float32), np.asarray(Wu0, np.float32)
    Wg1, Wu1 = np.asarray(Wg1, np.float32), np.asarray(Wu1, np.float32)
    Weg0, Weu0 = np.asarray(Weg0, np.float32), np.asarray(Weu0, np.float32)
    Weg1, Weu1 = np.asarray(Weg1, np.float32), np.asarray(Weu1, np.float32)
    agg_ea = np.zeros((N_PAD, CE), np.float32)
    np.add.at(agg_ea, dslot, ea * enorm[:, None])
    Wecat0 = np.concatenate([Weg0[:, HID:], Weu0], axis=1)   # [8, 64]
    Wecat1 = np.concatenate([Weg1[:, HID:], Weu1], axis=1)   # [8, 64]
    bcat0 = np.concatenate([np.asarray(bg0, np.float32)[HID:], np.asarray(bu0, np.float32)])
    bcat1 = np.concatenate([np.asarray(bg1, np.float32)[HID:], np.asarray(bu1, np.float32)])
    pb0 = agg_ea @ Wecat0 + bcat0[None, :]                   # [N_PAD, 64]
    pb1 = agg_ea @ Wecat1 + bcat1[None, :]
    for c in range(NCORES):
        s = c * NODES_PER_CORE
        # [128 off, NBLK, 64]
        per_core[c]['pb0'] = np.ascontiguousarray(
            pb0[s:s + NODES_PER_CORE].reshape(NBLK, 128, 64).transpose(1, 0, 2))
        per_core[c]['pb1'] = np.ascontiguousarray(
            pb1[s:s + NODES_PER_CORE].reshape(NBLK, 128, 64).transpose(1, 0, 2))

    # --- weights
    Wcat0 = np.concatenate([Wg0[:CIN, HID:], Wu0[:CIN]], axis=1)      # [16, 64]
    Wcat1 = np.concatenate([Wg1[:HID, HID:], Wu1[:HID]], axis=1)      # [32, 64]
    w0_bd = np.zeros((96, 384), np.float32)      # 6 t-blocks of [16, 64]
    for tt in range(6):
        w0_bd[tt * 16:(tt + 1) * 16, tt * 64:(tt + 1) * 64] = Wcat0
    w1_bd = np.zeros((96, 192), np.float32)      # 3 t-blocks of [32, 64]
    for tt in range(3):
        w1_bd[tt * 32:(tt + 1) * 32, tt * 64:(tt + 1) * 64] = Wcat1

    ident = _to_bf16(np.eye(128, dtype=np.float32))
    shared = dict(wcat0_rep=_to_bf16(w0_bd), wcat1_rep=_to_bf16(w1_bd),
                  ident=ident)
    gates_key = tuple(tuple(int(g) for g in row) for row in gates)
    bnd_key = tuple(bnd)
    return shared, per_core, perm_slot, C, gates_key, bnd_key


# ---------------------------------------------------------------- bass build
import os


def _build_nc(C, gates, bnd):
    E_blk = C * 128
    E_pad = NBLK * E_blk
    nsub = len(bnd) - 1
    nc = bacc.Bacc(None, target_bir_lowering=False, num_swdge_queues=4)

    xe_d = nc.declare_dram_parameter("xe", [128, NBLK * C, T * CIN], DT.bfloat16, isOutput=False)
    idx_d = nc.declare_dram_parameter("idx", [128, E_pad // 16], DT.int16, isOutput=False)
    sc_d = nc.declare_dram_parameter("sc", [128, NBLK * C, 128], DT.bfloat16, isOutput=False)
    w0_d = nc.declare_dram_parameter("wcat0_rep", [96, 384], DT.bfloat16, isOutput=False)
    w1_d = nc.declare_dram_parameter("wcat1_rep", [96, 192], DT.bfloat16, isOutput=False)
    pb0_d = nc.declare_dram_parameter("pb0", [128, NBLK, 64], DT.float32, isOutput=False)
    pb1_d = nc.declare_dram_parameter("pb1", [128, NBLK, 64], DT.float32, isOutput=False)
    ident_d = nc.declare_dram_parameter("ident", [128, 128], DT.bfloat16, isOutput=False)
    out_d = nc.declare_dram_parameter("out", [NODES_PER_CORE, T * HID], DT.float32, isOutput=True)

    h_table = nc.dram_tensor("h_table", [N_PAD, T * HID], DT.bfloat16,
                             addr_space="Shared")
    h_slices = [nc.dram_tensor(f"h_slice{g}", [gb * 128, T * HID], DT.bfloat16)
                for g, gb in enumerate(GROUPS)]

    with tile.TileContext(nc) as tc:
        with (
            tc.tile_pool(name="const", bufs=1) as constp,
            tc.tile_pool(name="big", bufs=1) as bigp,
            tc.tile_pool(name="msg1", bufs=NBLK) as msg1p,
            tc.tile_pool(name="work", bufs=2) as workp,
            tc.tile_pool(name="psum_cat", bufs=2, space="PSUM") as pcatp,
            tc.tile_pool(name="psum_mix", bufs=2, space="PSUM") as pmixp,
            tc.tile_pool(name="psum_xw", bufs=1, space="PSUM") as pxwp,
        ):
            ident_sb = constp.tile([128, 128], DT.bfloat16)
            nc.sync.dma_start(ident_sb[:], ident_d[:])

            # ---- constants / streamed inputs
            xe_sb = bigp.tile([128, NBLK * C, T * CIN], DT.bfloat16)
            sc_sb = bigp.tile([128, NBLK * C, 128], DT.bfloat16)
            for b in range(NBLK):
                nc.sync.dma_start(xe_sb[:, b * C:(b + 1) * C, :],
                                  xe_d[:, b * C:(b + 1) * C, :])
                nc.sync.dma_start(sc_sb[:, b * C:(b + 1) * C, :],
                                  sc_d[:, b * C:(b + 1) * C, :])
            idx_sb = constp.tile([128, E_pad // 16], DT.int16)
            nc.sync.dma_start(idx_sb[:], idx_d[:])
            w0_sb = constp.tile([96, 384], DT.bfloat16)
            nc.sync.dma_start(w0_sb[:], w0_d[:])
            w1_sb = constp.tile([96, 192], DT.bfloat16)
            nc.sync.dma_start(w1_sb[:], w1_d[:])
            pb0_sb = constp.tile([128, NBLK, 64], DT.float32)
            nc.sync.dma_start(pb0_sb[:], pb0_d[:])
            pb1_sb = constp.tile([128, NBLK, 64], DT.float32)
            nc.sync.dma_start(pb1_sb[:], pb1_d[:])

            gq = [0]

            def compute_block(lidx, b, rhs_tile, rhs_off):
                cw = CIN if lidx == 0 else HID        # channels per t
                fa = T * cw                           # agg width (192 / 384)
                nhalf = fa // 96                      # transpose halves (2 / 4)
                pcat = pcatp.tile([128, fa], DT.float32, tag="pcat")
                for ch in range(C):
                    nc.tensor.matmul(pcat[:], sc_sb[:, b * C + ch, :],
                                     rhs_tile[:, rhs_off + ch, 0:fa],
                                     start=(ch == 0), stop=(ch == C - 1))
                pb_sb = pb0_sb if lidx == 0 else pb1_sb
                agg_bf = workp.tile([128, fa], DT.bfloat16, tag="agg_bf")
                nc.vector.tensor_copy(agg_bf[:], pcat[:])
                aggT = workp.tile([96, nhalf, 128], DT.bfloat16, tag="aggT")
                for hh in range(nhalf):
                    pT = pmixp.tile([96, 128], DT.bfloat16, tag="pmix")
                    nc.tensor.transpose(pT[:], agg_bf[:, hh * 96:(hh + 1) * 96],
                                        ident_sb[:])
                    nc.vector.tensor_copy(aggT[:, hh, :], pT[:])
                pxw = pxwp.tile([128, 1024], DT.float32, tag="pxw")
                if lidx == 0:
                    nh, hstride, width, na, wsb = 2, 512, 384, 6, w0_sb
                else:
                    nh, hstride, width, na, wsb = 4, 256, 192, 3, w1_sb
                for hh in range(nh):
                    nc.tensor.matmul(
                        pxw[:, hh * hstride:hh * hstride + width],
                        aggT[:, hh, :], wsb[:], start=True, stop=True)
                xw_view = (pxw[:].rearrange("p (h x) -> p h x", h=nh)
                           [:, :, 0:width]
                           .rearrange("p h (a d) -> p h a d", d=64))
                pre = workp.tile([128, T, 64], DT.float32, tag="pre")
                nc.vector.tensor_add(
                    pre[:].rearrange("p (h a) d -> p h a d", h=nh),
                    xw_view,
                    pb_sb[:, b, :].unsqueeze(1).unsqueeze(1)
                    .broadcast_to((128, nh, na, 64)))
                oneR = workp.tile([128, T, 32], DT.float32, tag="oneR")
                nc.scalar.activation(oneR[:], pre[:, :, 0:32],
                                     mybir.ActivationFunctionType.Sigmoid,
                                     scale=-1.0)
                hc = workp.tile([128, T, 32], DT.float32, tag="hc")
                nc.scalar.activation(hc[:], pre[:, :, 32:64],
                                     mybir.ActivationFunctionType.Tanh)
                if lidx == 0:
                    # h = (1-R) * relu(HC), bf16 (raw; norms live in sc)
                    h_bf = workp.tile([128, T * HID], DT.bfloat16, tag="h_bf")
                    nc.vector.scalar_tensor_tensor(
                        h_bf[:].rearrange("p (t d) -> p t d", d=32),
                        hc[:], 0.0, oneR[:],
                        mybir.AluOpType.max, mybir.AluOpType.mult)
                    g = sum(1 for gs in GSTART if b >= gs) - 1
                    lb = b - GSTART[g]
                    nc.sync.dma_start(
                        h_slices[g][lb * 128:(lb + 1) * 128, :], h_bf[:])
                else:
                    o_sb = workp.tile([128, T * HID], DT.float32, tag="o_sb")
                    nc.vector.tensor_mul(
                        o_sb[:].rearrange("p (t d) -> p t d", d=32),
                        hc[:], oneR[:])
                    nc.sync.dma_start(out_d[b * 128:(b + 1) * 128, :], o_sb[:])

            # ---- layer 0 + h exchange: per-group AllGathers (each group has
            # its own input tensor so its trigger only waits on its blocks)
            gbase = [0]
            for gb in GROUPS:
                gbase.append(gbase[-1] + NCORES * gb * 128)
            for b in range(NBLK):
                compute_block(0, b, xe_sb, b * C)
                for g, gb in enumerate(GROUPS):
                    if b == GSTART[g] + gb - 1:
                        nc.gpsimd.collective_compute(
                            "AllGather", mybir.AluOpType.bypass,
                            replica_groups=[list(range(NCORES))],
                            ins=[h_slices[g][:]],
                            outs=[h_table[gbase[g]:gbase[g + 1], :]])

            # ---- layer-1 gathers: (gate, block, subrange) sorted by gate.
            # Issued after all AG triggers; each descgen waits only on the
            # AllGather chunks covering its source blocks.
            msgs1 = {}
            for b in range(NBLK):
                msgs1[b] = msg1p.tile([128, C, T * HID], DT.bfloat16,
                                      tag="m1", name=f"msg1_{b}")
            order = sorted((gates[b][s], b, s)
                           for b in range(NBLK) for s in range(nsub))
            for g, b, s in order:
                lo, hi = bnd[s], bnd[s + 1]
                nrow = (hi - lo) * 128
                # prefix-slice the source so the dep tracker only gates this
                # sub-gather on the AllGather groups 0..g it actually reads
                src_ap = h_table[0:gbase[g + 1], :]
                nc.gpsimd.dma_gather(
                    msgs1[b][:, lo:hi, :], src_ap,
                    idx_sb[:, (b * E_blk + lo * 128) // 16:(b * E_blk + hi * 128) // 16],
                    nrow, nrow, T * HID, single_packet=True,
                    queue_num=gq[0] % 4)
                gq[0] += 1

            # ---- layer 1
            for b in range(NBLK):
                compute_block(1, b, msgs1[b], 0)

    nc.compile()
    return nc


_NC_CACHE = {}
_LAST_RESULT = None


def kernel(**inputs) -> np.ndarray:
    shared, per_core, perm_slot, C, gates, bnd = _host_prep(**inputs)
    key = (C, gates, bnd)
    if key not in _NC_CACHE:
        _NC_CACHE[key] = _build_nc(C, gates, bnd)
    nc = _NC_CACHE[key]
    in_maps = []
    for c in range(NCORES):
        m = dict(
            xe=np.ascontiguousarray(per_core[c]['xe']),
            idx=np.ascontiguousarray(per_core[c]['idx']),
            sc=np.ascontiguousarray(per_core[c]['sc']),
            pb0=per_core[c]['pb0'], pb1=per_core[c]['pb1'],
            wcat0_rep=shared['wcat0_rep'], wcat1_rep=shared['wcat1_rep'],
            ident=shared['ident'],
        )
        in_maps.append(m)
    trace = bool(os.environ.get('KTRACE'))
    if trace:
        try:
            import ntff_shim  # registers the axon NTFF profile hook
        except Exception:
            pass
    res = run_bass_kernel_spmd(nc, in_maps, core_ids=list(range(NCORES)),
                               trace=trace)
    global _LAST_RESULT
    _LAST_RESULT = res
    out_pad = np.concatenate([res.results[c]["out"] for c in range(NCORES)], axis=0)
    out = out_pad[perm_slot].reshape(N, T, HID).transpose(1, 0, 2)
    return np.ascontiguousarray(out.astype(np.float32))


if __name__ == "__main__":
    pass
